# revision 10
# baseline (speedup 1.0000x reference)
"""Causal self-attention with interleaved RoPE on 8 Trainium2 NeuronCores.

Problem: B=4, T=2048, C=1024, H=16, D=64 (fp32).
  qkv = x @ W_in + b_in ; per-head interleaved RoPE on q,k ;
  causal softmax attention ; y @ W_out + b_out.

Sharding: core c <-> (batch b = c//2, head-half = c%2, 8 heads each).
Per core: QKV projection for its heads, attention, then an 8-core
AllGather exchanges (unnormalized) per-head attention outputs + softmax
row sums; each core assembles all 16 heads for its (batch, T-half),
normalizes, and computes the output projection for a disjoint output
slice. Matmuls run as float32r (~1e-4 rel err, 4x fp32 rate).

Layout notes:
 - x is passed transposed (xT [C, T]) with a ones row appended so b_in
   rides the contraction (K = C+1).
 - W_in q/k columns are permuted per head to de-interleave RoPE pairs
   (evens then odds); RoPE becomes q*cos2 + swap32(q)*sin2 where swap32
   swaps 32-col halves within each 64-col head group. S = q.k is
   invariant to the (shared) permutation.
 - Scores are computed transposed (S^T [tk, tq]) so softmax(P^T) feeds
   P@V directly as the moving operand, with no P transposes. exp() is
   applied without max-subtraction (|S|*scale <= ~6 for randn inputs,
   safely inside fp32 exp range); row sums come free via ones columns
   appended to V (out partition rows 64..71 of the PV accumulation).
"""

import numpy as np

B, T, C, H = 4, 2048, 1024, 16
D = C // H            # 64
HPC = H // 2          # heads per core = 8
N_CORES = 8
ROPE_BASE = 10000.0
TB = T // 128         # 16 t-blocks
THALF = T // 2        # 1024

_CACHE = {}


def _build_program():
    import concourse.bass as bass
    import concourse.bacc as bacc
    import concourse.tile as tile
    import concourse.mybir as mybir

    f32 = mybir.dt.float32
    f32r = mybir.dt.float32r
    i32 = mybir.dt.int32

    nc = bacc.Bacc("TRN2", target_bir_lowering=False, debug=False,
                   num_devices=N_CORES)

    xt_d = nc.dram_tensor("xt", [C + 1, T], f32r, kind="ExternalInput")
    wqk_d = nc.dram_tensor("wqk", [C + 1, 2 * HPC * D], f32r, kind="ExternalInput")
    wv_d = nc.dram_tensor("wv", [C + 1, HPC * D], f32r, kind="ExternalInput")
    wout_d = nc.dram_tensor("wout", [C, C], f32r, kind="ExternalInput")
    cos2_d = nc.dram_tensor("cos2", [T, D], f32, kind="ExternalInput")
    sin2_d = nc.dram_tensor("sin2", [T, D], f32, kind="ExternalInput")
    tri_d = nc.dram_tensor("trimask", [128, 128], f32r, kind="ExternalInput")
    id_d = nc.dram_tensor("ident", [128, 128], f32r, kind="ExternalInput")
    ones_d = nc.dram_tensor("ones64", [128, 64], f32r, kind="ExternalInput")
    gidx_d = nc.dram_tensor("gidx", [128, 8], i32, kind="ExternalInput")
    gsidx_d = nc.dram_tensor("gsidx", [128, 8], i32, kind="ExternalInput")
    out_d = nc.dram_tensor("out", [THALF, C], f32, kind="ExternalOutput")

    AG_ROWS = HPC * D + HPC  # 512 yT rows + 8 sums rows = 520

    with tile.TileContext(nc) as tc:
        with (
            tc.tile_pool(name="g", bufs=1) as g,
            tc.tile_pool(name="psa", bufs=2, space="PSUM") as psa,   # 1-bank: transp/st
            tc.tile_pool(name="psb", bufs=2, space="PSUM") as psb,   # 2-bank: qk/proj
            tc.tile_pool(name="psc", bufs=2, space="PSUM") as psc,   # 1-bank: v/ot
            tc.tile_pool(name="dram", bufs=1, space="DRAM") as dram,
        ):
            # ---- constants ----
            cos_sb = g.tile([128, TB, D], f32)
            sin_sb = g.tile([128, TB, D], f32)
            nc.sync.dma_start(cos_sb[:], cos2_d[:].rearrange("(tb p) j -> p tb j", p=128))
            nc.sync.dma_start(sin_sb[:], sin2_d[:].rearrange("(tb p) j -> p tb j", p=128))
            tri_sb = g.tile([128, 128], f32r)
            nc.sync.dma_start(tri_sb[:], tri_d[:])
            id_sb = g.tile([128, 128], f32r)
            nc.sync.dma_start(id_sb[:], id_d[:])
            ones_sb = g.tile([128, 64], f32r)
            nc.sync.dma_start(ones_sb[:], ones_d[:])
            gidx_sb = g.tile([128, 8], i32)
            nc.sync.dma_start(gidx_sb[:], gidx_d[:])
            gsidx_sb = g.tile([128, 8], i32)
            nc.sync.dma_start(gsidx_sb[:], gsidx_d[:])

            # ---- persistent activations ----
            qt_sb = g.tile([128, 4, T], f32r)      # [head-pair rows, pair, t]
            kt_sb = g.tile([128, 4, T], f32r)
            v_sb = g.tile([128, TB, HPC, 72], f32r)  # [tk in chunk, chunk, head, d+ones]
            for tb in range(TB):
                # ones columns 64..71 for every head (rows 64.. of PV psum = row sums)
                nc.sync.dma_start(
                    v_sb[:, tb, :, 64:72],
                    ones_sb[:].rearrange("p (h j) -> p h j", h=8),
                )

            ag_in = dram.tile([AG_ROWS, T], f32r)

            # ================= Phase B: QKV projection + RoPE + transposes ====
            with (
                tc.tile_pool(name="wts", bufs=1) as wts,
                tc.tile_pool(name="xp", bufs=2) as xp,
                tc.tile_pool(name="rp", bufs=3) as rp,
            ):
                wqk_sb = wts.tile([128, 8, 1024], f32r)
                nc.sync.dma_start(wqk_sb[:], wqk_d[0:C, :].rearrange("(kc p) n -> p kc n", p=128))
                wqkb_sb = wts.tile([1, 1024], f32r)
                nc.sync.dma_start(wqkb_sb[:], wqk_d[C:C + 1, :])
                wv_sb = wts.tile([128, 8, 512], f32r)
                nc.sync.dma_start(wv_sb[:], wv_d[0:C, :].rearrange("(kc p) n -> p kc n", p=128))
                wvb_sb = wts.tile([1, 512], f32r)
                nc.sync.dma_start(wvb_sb[:], wv_d[C:C + 1, :])

                for tb in range(TB):
                    ts = slice(tb * 128, (tb + 1) * 128)
                    xt_t = xp.tile([128, 8, 128], f32r, tag="xt")
                    nc.sync.dma_start(xt_t[:], xt_d[0:C, ts].rearrange("(kc p) t -> p kc t", p=128))
                    xtb_t = xp.tile([1, 128], f32r, tag="xtb")
                    nc.sync.dma_start(xtb_t[:], xt_d[C:C + 1, ts])

                    qk_ps = psb.tile([128, 1024], f32, tag="b2")
                    v_ps = psc.tile([128, 512], f32, tag="b1")
                    for kc in range(8):
                        st = (kc == 0)
                        nc.tensor.matmul(qk_ps[:, 0:512], xt_t[:, kc, :],
                                         wqk_sb[:, kc, 0:512], start=st, stop=False)
                        nc.tensor.matmul(qk_ps[:, 512:1024], xt_t[:, kc, :],
                                         wqk_sb[:, kc, 512:1024], start=st, stop=False)
                        nc.tensor.matmul(v_ps[:], xt_t[:, kc, :],
                                         wv_sb[:, kc, :], start=st, stop=False)
                    nc.tensor.matmul(qk_ps[:, 0:512], xtb_t[:], wqkb_sb[:, 0:512],
                                     start=False, stop=True)
                    nc.tensor.matmul(qk_ps[:, 512:1024], xtb_t[:], wqkb_sb[:, 512:1024],
                                     start=False, stop=True)
                    nc.tensor.matmul(v_ps[:], xtb_t[:], wvb_sb[:], start=False, stop=True)

                    # RoPE on q and k at once: qkr = qk*cos2 + swap32(qk)*sin2
                    qk3 = qk_ps[:].rearrange("p (a j) -> p a j", j=64)
                    swap = bass.AP(
                        tensor=qk3.tensor,
                        offset=qk3.offset + 32,
                        ap=[qk3.ap[0], [64, 16], [-32, 2], [1, 32]],
                    )
                    _cs = cos_sb[:, tb, :]
                    cosb = bass.AP(tensor=_cs.tensor, offset=_cs.offset,
                                   ap=[_cs.ap[0], [0, 16], [1, 64]])
                    _sn = sin_sb[:, tb, :]
                    sinb = bass.AP(tensor=_sn.tensor, offset=_sn.offset,
                                   ap=[_sn.ap[0], [0, 16], [1, 64]])
                    t1 = rp.tile([128, 1024], f32, tag="t1", bufs=2)
                    qkr = rp.tile([128, 1024], f32r, tag="qkr")
                    nc.vector.tensor_mul(t1[:].rearrange("p (a j) -> p a j", j=64), swap, sinb)
                    nc.vector.tensor_mul(qkr[:].rearrange("p (a j) -> p a j", j=64), qk3, cosb)
                    nc.vector.tensor_add(qkr[:], qkr[:], t1[:])

                    # v -> v_sb
                    nc.scalar.copy(v_sb[:, tb, :, 0:64],
                                   v_ps[:].rearrange("p (h d) -> p h d", h=8))

                    # transposes: per head pair [128t x 128cols] -> [128cols x 128t]
                    for pp in range(4):
                        tq_ps = psa.tile([128, 128], f32r, tag="a1")
                        nc.tensor.transpose(tq_ps[:], qkr[:, pp * 128:(pp + 1) * 128], id_sb[:])
                        nc.scalar.copy(qt_sb[:, pp, ts], tq_ps[:])
                        tk_ps = psa.tile([128, 128], f32r, tag="a1")
                        nc.tensor.transpose(tk_ps[:], qkr[:, 512 + pp * 128:512 + (pp + 1) * 128], id_sb[:])
                        nc.scalar.copy(kt_sb[:, pp, ts], tk_ps[:])

            # ================= Phase C: attention ============================
            SCALE = 1.0 / float(np.sqrt(D))
            with (
                tc.tile_pool(name="pP", bufs=4) as pP,
                tc.tile_pool(name="tmpp", bufs=3) as tmpp,
            ):
                _phase_c(nc, tc, bass, mybir, psa, psc, pP, tmpp,
                         qt_sb, kt_sb, v_sb, tri_sb, ag_in, SCALE, f32, f32r)

            # ================= Phase D: exchange + output projection =========
            ag_out = dram.tile([N_CORES, AG_ROWS, T], f32r, addr_space="Shared",
                               name="ag_out")
            nc.gpsimd.collective_compute(
                "AllGather",
                bass.mybir.AluOpType.bypass,
                ins=[ag_in.opt()],
                outs=[ag_out.opt()],
                replica_groups=[list(range(N_CORES))],
            )
            ag_flat = ag_out[:].rearrange("r a (s n) -> (r a s) n", s=2)

            with tc.tile_pool(name="dp", bufs=1) as dp:
                wout_sb = dp.tile([128, 8, 1024], f32r)
                nc.sync.dma_start(wout_sb[:], wout_d[:].rearrange("(kc p) n -> p kc n", p=128))

                scaled = dp.tile([128, 8, 1024], f32r)
                for k in range(8):
                    yc = dp.tile([128, 1024], f32r, tag="yc", bufs=2)
                    nc.gpsimd.indirect_dma_start(
                        out=yc[:], out_offset=None,
                        in_=ag_flat,
                        in_offset=bass.IndirectOffsetOnAxis(ap=gidx_sb[:, k:k + 1], axis=0),
                    )
                    # per-chunk row sums, pre-broadcast to 128 partitions by
                    # gathering with repeated indices (head 2k rows 0-63,
                    # head 2k+1 rows 64-127)
                    srow = dp.tile([128, 1024], f32r, tag="srow", bufs=2)
                    nc.gpsimd.indirect_dma_start(
                        out=srow[:], out_offset=None,
                        in_=ag_flat,
                        in_offset=bass.IndirectOffsetOnAxis(ap=gsidx_sb[:, k:k + 1], axis=0),
                    )
                    rrep = dp.tile([128, 1024], f32, tag="rrep", bufs=2)
                    nc.vector.reciprocal(rrep[:], srow[:].bitcast(f32))
                    nc.vector.tensor_mul(scaled[:, k, :], yc[:], rrep[:])

                for tb2 in range(8):
                    pr_ps = psb.tile([128, 1024], f32, tag="b2")
                    for k in range(8):
                        nc.tensor.matmul(pr_ps[:, 0:512],
                                         scaled[:, k, tb2 * 128:(tb2 + 1) * 128],
                                         wout_sb[:, k, 0:512],
                                         start=(k == 0), stop=(k == 7))
                        nc.tensor.matmul(pr_ps[:, 512:1024],
                                         scaled[:, k, tb2 * 128:(tb2 + 1) * 128],
                                         wout_sb[:, k, 512:1024],
                                         start=(k == 0), stop=(k == 7))
                    o_t = dp.tile([128, 1024], f32, tag="ot", bufs=2)
                    nc.scalar.copy(o_t[:], pr_ps[:])
                    nc.sync.dma_start(out_d[tb2 * 128:(tb2 + 1) * 128, :], o_t[:])

    nc.compile()
    return nc


def _phase_c(nc, tc, bass, mybir, psa, psc, pP, tmpp,
             qt_sb, kt_sb, v_sb, tri_sb, ag_in, SCALE, f32, f32r):
    for J in range(4):
        js = slice(J * 512, (J + 1) * 512)
        for h in range(HPC):
            pp, row = h // 2, (h % 2) * 64
            ot_ps = psc.tile([128, 512], f32, tag="b1")
            n_i = 4 * J + 4
            for i in range(n_i):
                d0 = max(0, (i - 4 * J) * 128)
                st_ps = psa.tile([128, 512], f32, tag="a1")
                nc.tensor.matmul(
                    st_ps[:, d0:512],
                    kt_sb[row:row + 64, pp, i * 128:(i + 1) * 128],
                    qt_sb[row:row + 64, pp, J * 512 + d0:(J + 1) * 512],
                    start=True, stop=True,
                )
                p_t = pP.tile([128, 512], f32r, tag="p")
                nc.scalar.activation(p_t[:, d0:512], st_ps[:, d0:512],
                                     mybir.ActivationFunctionType.Exp,
                                     scale=SCALE)
                if i >= 4 * J:
                    nc.vector.tensor_mul(p_t[:, d0:d0 + 128],
                                         p_t[:, d0:d0 + 128], tri_sb[:])
                nc.tensor.matmul(ot_ps[0:72, d0:512], v_sb[:, i, h, 0:72],
                                 p_t[:, d0:512],
                                 start=(i == 0), stop=(i == n_i - 1))
            tmp_t = tmpp.tile([128, 512], f32r, tag="tmp")
            nc.vector.tensor_copy(tmp_t[0:72, :], ot_ps[0:72, :])
            nc.sync.dma_start(ag_in[h * 64:(h + 1) * 64, js], tmp_t[0:64, :])
            nc.sync.dma_start(ag_in[512 + h:513 + h, js], tmp_t[64:65, :])


def _host_prep(x, W_in, b_in, W_out):
    """Build per-core input maps."""
    perm = np.concatenate([np.arange(0, D, 2), np.arange(1, D, 2)])  # de-interleave
    inv_freq = 1.0 / (ROPE_BASE ** (np.arange(0, D, 2, dtype=np.float64) / D))
    tpos = np.arange(T, dtype=np.float64)
    freqs = np.outer(tpos, inv_freq)                   # [T, 32]
    cosw = np.cos(freqs).astype(np.float32)
    sinw = np.sin(freqs).astype(np.float32)
    cos2 = np.concatenate([cosw, cosw], axis=1)        # [T, 64]
    sin2 = np.concatenate([-sinw, sinw], axis=1)       # [T, 64]

    tri = (np.arange(128)[None, :] >= np.arange(128)[:, None]).astype(np.float32)
    ident = np.eye(128, dtype=np.float32)
    ones64 = np.ones((128, 64), dtype=np.float32)

    in_maps = []
    for c in range(N_CORES):
        b, half = c // 2, c % 2
        heads = np.arange(half * HPC, (half + 1) * HPC)

        xt = np.concatenate([np.ascontiguousarray(x[b].T),
                             np.ones((1, T), np.float32)], axis=0)

        qcols = np.concatenate([h * D + perm for h in heads])            # q block
        kcols = np.concatenate([C + h * D + perm for h in heads])        # k block
        vcols = np.concatenate([2 * C + h * D + np.arange(D) for h in heads])
        wqk = np.concatenate([W_in[:, np.concatenate([qcols, kcols])],
                              b_in[None, np.concatenate([qcols, kcols])]], axis=0)
        wv = np.concatenate([W_in[:, vcols], b_in[None, vcols]], axis=0)

        gidx = np.empty((128, 8), np.int32)
        for k in range(8):
            gc = k * 128 + np.arange(128)
            rank = 2 * b + gc // 512
            r = gc % 512
            gidx[:, k] = rank * 1040 + r * 2 + half
        gsidx = np.empty((128, 8), np.int32)
        for k in range(8):
            hh = 2 * k + np.arange(128) // 64       # head per partition
            rank = 2 * b + hh // 8
            gsidx[:, k] = rank * 1040 + (512 + hh % 8) * 2 + half

        in_maps.append({
            "xt": np.ascontiguousarray(xt),
            "wqk": np.ascontiguousarray(wqk.astype(np.float32)),
            "wv": np.ascontiguousarray(wv.astype(np.float32)),
            "wout": np.ascontiguousarray(W_out.astype(np.float32)),
            "cos2": cos2, "sin2": sin2,
            "trimask": tri, "ident": ident, "ones64": ones64,
            "gidx": gidx, "gsidx": gsidx,
        })
    return in_maps


LAST_RESULT = None


def kernel(x, W_in, b_in, W_out, b_out, _trace=False):
    global LAST_RESULT
    from concourse.bass_utils import run_bass_kernel_spmd

    x = np.asarray(x, dtype=np.float32)
    W_in = np.asarray(W_in, dtype=np.float32)
    b_in = np.asarray(b_in, dtype=np.float32)
    W_out = np.asarray(W_out, dtype=np.float32)
    b_out = np.asarray(b_out, dtype=np.float32)

    if "nc" not in _CACHE:
        _CACHE["nc"] = _build_program()
    nc = _CACHE["nc"]

    in_maps = _host_prep(x, W_in, b_in, W_out)
    res = run_bass_kernel_spmd(nc, in_maps, core_ids=list(range(N_CORES)),
                               trace=_trace)
    LAST_RESULT = res

    out = np.empty((B, T, C), np.float32)
    for c in range(N_CORES):
        b, half = c // 2, c % 2
        out[b, half * THALF:(half + 1) * THALF, :] = res.results[c]["out"]
    if np.any(b_out != 0):
        out = out + b_out[None, None, :]
    return out


# revision 11
# speedup vs baseline: 1.2144x; 1.2144x over previous
"""Causal self-attention with interleaved RoPE on 8 Trainium2 NeuronCores.

Problem: B=4, T=2048, C=1024, H=16, D=64 (fp32).
  qkv = x @ W_in + b_in ; per-head interleaved RoPE on q,k ;
  causal softmax attention ; y @ W_out + b_out.

Sharding: core c <-> (batch b = c//2, head-half = c%2, 8 heads each).
Per core: QKV projection for its heads (float32r matmuls, ~1e-4 rel
err at 4x fp32 rate), attention in bf16 (fp32 PSUM accumulation), then
per-head-pair AllGathers exchange unnormalized attention outputs +
softmax row sums across all 8 cores; each core assembles all 16 heads
for its (batch, T-half), normalizes, and computes the output
projection (f32r) for a disjoint output slice.

Layout notes:
 - x is passed transposed (xT [C, T]) with a ones row appended so b_in
   rides the contraction (K = C+1).
 - W_in q/k columns are permuted per head to de-interleave RoPE pairs
   (evens then odds); RoPE becomes q*cos2 + swap32(q)*sin2 where swap32
   swaps 32-col halves within each 64-col head group. S = q.k is
   invariant to the (shared) permutation.
 - Scores are computed transposed (S^T [tk, tq]) so softmax(P^T) feeds
   P@V directly as the moving operand, with no P transposes. exp() is
   applied without max-subtraction (|S|*scale <= ~6 for randn inputs,
   safely inside fp32 exp range); row sums come free via ones columns
   appended to V (rows 64..71 of the PV accumulation).
 - Normalization happens after the exchange: row-sum rows are gathered
   with repeated indices (a broadcast gather), reciprocals multiply the
   gathered yT chunks before the output projection.
"""

import numpy as np

B, T, C, H = 4, 2048, 1024, 16
D = C // H            # 64
HPC = H // 2          # heads per core = 8
N_CORES = 8
ROPE_BASE = 10000.0
TB = T // 128         # 16 t-blocks
THALF = T // 2        # 1024

_CACHE = {}


def _build_program():
    import concourse.bass as bass
    import concourse.bacc as bacc
    import concourse.tile as tile
    import concourse.mybir as mybir

    f32 = mybir.dt.float32
    f32r = mybir.dt.float32r
    bf16 = mybir.dt.bfloat16
    i32 = mybir.dt.int32

    nc = bacc.Bacc("TRN2", target_bir_lowering=False, debug=False,
                   num_devices=N_CORES)

    xt_d = nc.dram_tensor("xt", [C + 1, T], f32r, kind="ExternalInput")
    wqk_d = nc.dram_tensor("wqk", [C + 1, 2 * HPC * D], f32r, kind="ExternalInput")
    wv_d = nc.dram_tensor("wv", [C + 1, HPC * D], f32r, kind="ExternalInput")
    wout_d = nc.dram_tensor("wout", [C, C], f32r, kind="ExternalInput")
    cos2_d = nc.dram_tensor("cos2", [T, D], f32, kind="ExternalInput")
    sin2_d = nc.dram_tensor("sin2", [T, D], f32, kind="ExternalInput")
    tri_d = nc.dram_tensor("trimask", [128, 128], bf16, kind="ExternalInput")
    id_d = nc.dram_tensor("ident", [128, 128], bf16, kind="ExternalInput")
    ones_d = nc.dram_tensor("ones64", [128, 64], bf16, kind="ExternalInput")
    gidx_d = nc.dram_tensor("gidx", [128, 8], i32, kind="ExternalInput")
    gsidx_d = nc.dram_tensor("gsidx", [128, 8], i32, kind="ExternalInput")
    out_d = nc.dram_tensor("out", [THALF, C], f32, kind="ExternalOutput")

    AGR = 130  # per-pair AG rows: 2 heads x 64 yT + 2 sums

    with tile.TileContext(nc) as tc:
        with (
            tc.tile_pool(name="g", bufs=1) as g,
            tc.tile_pool(name="psa", bufs=3, space="PSUM") as psa,   # st / transp
            tc.tile_pool(name="psb", bufs=3, space="PSUM") as psb,   # q/k proj, out proj
            tc.tile_pool(name="psc", bufs=2, space="PSUM") as psc,   # v proj / ot
            tc.tile_pool(name="dram", bufs=1, space="DRAM") as dram,
        ):
            # ---- constants ----
            cos_sb = g.tile([128, TB, D], f32)
            sin_sb = g.tile([128, TB, D], f32)
            nc.sync.dma_start(cos_sb[:], cos2_d[:].rearrange("(tb p) j -> p tb j", p=128))
            nc.sync.dma_start(sin_sb[:], sin2_d[:].rearrange("(tb p) j -> p tb j", p=128))
            tri_sb = g.tile([128, 128], bf16)
            nc.sync.dma_start(tri_sb[:], tri_d[:])
            id_sb = g.tile([128, 128], bf16)
            nc.sync.dma_start(id_sb[:], id_d[:])
            ones_sb = g.tile([128, 64], bf16)
            nc.sync.dma_start(ones_sb[:], ones_d[:])
            gidx_sb = g.tile([128, 8], i32)
            nc.sync.dma_start(gidx_sb[:], gidx_d[:])
            gsidx_sb = g.tile([128, 8], i32)
            nc.sync.dma_start(gsidx_sb[:], gsidx_d[:])

            # ---- persistent activations (bf16) ----
            qt_sb = g.tile([128, 4, T], bf16)      # [pair rows, pair, t]
            kt_sb = g.tile([128, 4, T], bf16)
            v_sb = g.tile([128, TB, HPC, 72], bf16)
            for tb in range(TB):
                nc.sync.dma_start(
                    v_sb[:, tb, :, 64:72],
                    ones_sb[:].rearrange("p (h j) -> p h j", h=8),
                )

            ag_in = [dram.tile([AGR, T], bf16, name=f"ag_in{p}") for p in range(4)]
            ag_out = [dram.tile([N_CORES, AGR, T], bf16, addr_space="Shared",
                                name=f"ag_out{p}") for p in range(4)]

            # ================= Phase B: QKV projection + RoPE + transposes ====
            with (
                tc.tile_pool(name="wts", bufs=1) as wts,
                tc.tile_pool(name="xp", bufs=3) as xp,
                tc.tile_pool(name="rp", bufs=3) as rp,
            ):
                wqk_sb = wts.tile([128, 8, 1024], f32r)
                nc.sync.dma_start(wqk_sb[:], wqk_d[0:C, :].rearrange("(kc p) n -> p kc n", p=128))
                wqkb_sb = wts.tile([1, 1024], f32r)
                nc.sync.dma_start(wqkb_sb[:], wqk_d[C:C + 1, :])
                wv_sb = wts.tile([128, 8, 512], f32r)
                nc.sync.dma_start(wv_sb[:], wv_d[0:C, :].rearrange("(kc p) n -> p kc n", p=128))
                wvb_sb = wts.tile([1, 512], f32r)
                nc.sync.dma_start(wvb_sb[:], wv_d[C:C + 1, :])

                for tb in range(TB):
                    ts = slice(tb * 128, (tb + 1) * 128)
                    xt_t = xp.tile([128, 8, 128], f32r, tag="xt")
                    nc.sync.dma_start(xt_t[:], xt_d[0:C, ts].rearrange("(kc p) t -> p kc t", p=128))
                    xtb_t = xp.tile([1, 128], f32r, tag="xtb")
                    nc.sync.dma_start(xtb_t[:], xt_d[C:C + 1, ts])

                    q_ps = psb.tile([128, 512], f32, tag="b")
                    k_ps = psb.tile([128, 512], f32, tag="b")
                    v_ps = psc.tile([128, 512], f32, tag="c")
                    for kc in range(8):
                        st = (kc == 0)
                        nc.tensor.matmul(q_ps[:], xt_t[:, kc, :],
                                         wqk_sb[:, kc, 0:512], start=st, stop=False)
                        nc.tensor.matmul(k_ps[:], xt_t[:, kc, :],
                                         wqk_sb[:, kc, 512:1024], start=st, stop=False)
                        nc.tensor.matmul(v_ps[:], xt_t[:, kc, :],
                                         wv_sb[:, kc, :], start=st, stop=False)
                    nc.tensor.matmul(q_ps[:], xtb_t[:], wqkb_sb[:, 0:512],
                                     start=False, stop=True)
                    nc.tensor.matmul(k_ps[:], xtb_t[:], wqkb_sb[:, 512:1024],
                                     start=False, stop=True)
                    nc.tensor.matmul(v_ps[:], xtb_t[:], wvb_sb[:], start=False, stop=True)

                    # RoPE: r = x*cos2 + swap32(x)*sin2 (per 64-col head group)
                    qkr = rp.tile([128, 1024], bf16, tag="qkr")
                    _cs = cos_sb[:, tb, :]
                    cosb = bass.AP(tensor=_cs.tensor, offset=_cs.offset,
                                   ap=[_cs.ap[0], [0, 8], [1, 64]])
                    _sn = sin_sb[:, tb, :]
                    sinb = bass.AP(tensor=_sn.tensor, offset=_sn.offset,
                                   ap=[_sn.ap[0], [0, 8], [1, 64]])
                    for half, h_ps in ((0, q_ps), (1, k_ps)):
                        ps3 = h_ps[:].rearrange("p (a j) -> p a j", j=64)
                        swap = bass.AP(
                            tensor=ps3.tensor,
                            offset=ps3.offset + 32,
                            ap=[ps3.ap[0], [64, 8], [-32, 2], [1, 32]],
                        )
                        t1 = rp.tile([128, 512], f32, tag="t1", bufs=2)
                        dst = qkr[:, half * 512:(half + 1) * 512]
                        nc.vector.tensor_mul(t1[:].rearrange("p (a j) -> p a j", j=64),
                                             swap, sinb)
                        nc.vector.tensor_mul(dst.rearrange("p (a j) -> p a j", j=64),
                                             ps3, cosb)
                        nc.vector.tensor_add(dst, dst, t1[:])

                    nc.scalar.copy(v_sb[:, tb, :, 0:64],
                                   v_ps[:].rearrange("p (h d) -> p h d", h=8))

                    for pp in range(4):
                        tq_ps = psa.tile([128, 128], bf16, tag="a")
                        nc.tensor.transpose(tq_ps[:], qkr[:, pp * 128:(pp + 1) * 128], id_sb[:])
                        nc.scalar.copy(qt_sb[:, pp, ts], tq_ps[:])
                        tk_ps = psa.tile([128, 128], bf16, tag="a")
                        nc.tensor.transpose(tk_ps[:], qkr[:, 512 + pp * 128:512 + (pp + 1) * 128], id_sb[:])
                        nc.scalar.copy(kt_sb[:, pp, ts], tk_ps[:])

            # ================= Phase C: attention (pair-major) + AGs =========
            SCALE = 1.0 / float(np.sqrt(D))
            with (
                tc.tile_pool(name="pP", bufs=6) as pP,
                tc.tile_pool(name="tmpp", bufs=3) as tmpp,
            ):
                for pp in range(4):
                    for hh in range(2):
                        h = 2 * pp + hh
                        row = hh * 64
                        for J in range(4):
                            js = slice(J * 512, (J + 1) * 512)
                            ot_ps = psc.tile([128, 512], f32, tag="c")
                            n_i = 4 * J + 4
                            for i in range(n_i):
                                d0 = max(0, (i - 4 * J) * 128)
                                st_ps = psa.tile([128, 512], f32, tag="a")
                                nc.tensor.matmul(
                                    st_ps[:, d0:512],
                                    kt_sb[row:row + 64, pp, i * 128:(i + 1) * 128],
                                    qt_sb[row:row + 64, pp, J * 512 + d0:(J + 1) * 512],
                                    start=True, stop=True,
                                )
                                p_t = pP.tile([128, 512], bf16, tag="p")
                                nc.scalar.activation(p_t[:, d0:512], st_ps[:, d0:512],
                                                     mybir.ActivationFunctionType.Exp,
                                                     scale=SCALE)
                                if i >= 4 * J:
                                    nc.vector.tensor_mul(p_t[:, d0:d0 + 128],
                                                         p_t[:, d0:d0 + 128], tri_sb[:])
                                nc.tensor.matmul(ot_ps[0:72, d0:512], v_sb[:, i, h, 0:72],
                                                 p_t[:, d0:512],
                                                 start=(i == 0), stop=(i == n_i - 1))
                            tmp_t = tmpp.tile([128, 512], bf16, tag="tmp")
                            nc.vector.tensor_copy(tmp_t[0:72, :], ot_ps[0:72, :])
                            nc.sync.dma_start(ag_in[pp][row:row + 64, js], tmp_t[0:64, :])
                            nc.sync.dma_start(ag_in[pp][128 + hh:129 + hh, js], tmp_t[64:65, :])
                    nc.gpsimd.collective_compute(
                        "AllGather",
                        bass.mybir.AluOpType.bypass,
                        ins=[ag_in[pp].opt()],
                        outs=[ag_out[pp].opt()],
                        replica_groups=[list(range(N_CORES))],
                    )

            # ================= Phase D: gather + normalize + out-proj ========
            with tc.tile_pool(name="dp", bufs=1) as dp:
                wout_sb = dp.tile([128, 8, 1024], f32r)
                nc.sync.dma_start(wout_sb[:], wout_d[:].rearrange("(kc p) n -> p kc n", p=128))

                scaled = dp.tile([128, 8, 1024], f32r)
                for k in range(8):
                    ag_flat = ag_out[k % 4][:].rearrange("r a (s n) -> (r a s) n", s=2)
                    yc = dp.tile([128, 1024], bf16, tag="yc", bufs=2)
                    nc.gpsimd.indirect_dma_start(
                        out=yc[:], out_offset=None,
                        in_=ag_flat,
                        in_offset=bass.IndirectOffsetOnAxis(ap=gidx_sb[:, k:k + 1], axis=0),
                    )
                    srow = dp.tile([128, 1024], bf16, tag="srow", bufs=2)
                    nc.gpsimd.indirect_dma_start(
                        out=srow[:], out_offset=None,
                        in_=ag_flat,
                        in_offset=bass.IndirectOffsetOnAxis(ap=gsidx_sb[:, k:k + 1], axis=0),
                    )
                    rrep = dp.tile([128, 1024], f32, tag="rrep", bufs=2)
                    nc.vector.reciprocal(rrep[:], srow[:])
                    nc.vector.tensor_mul(scaled[:, k, :], yc[:], rrep[:])

                for tb2 in range(8):
                    pr0 = psb.tile([128, 512], f32, tag="b")
                    pr1 = psb.tile([128, 512], f32, tag="b")
                    for k in range(8):
                        nc.tensor.matmul(pr0[:],
                                         scaled[:, k, tb2 * 128:(tb2 + 1) * 128],
                                         wout_sb[:, k, 0:512],
                                         start=(k == 0), stop=(k == 7))
                        nc.tensor.matmul(pr1[:],
                                         scaled[:, k, tb2 * 128:(tb2 + 1) * 128],
                                         wout_sb[:, k, 512:1024],
                                         start=(k == 0), stop=(k == 7))
                    o_t = dp.tile([128, 1024], f32, tag="ot", bufs=3)
                    nc.scalar.copy(o_t[:, 0:512], pr0[:])
                    nc.scalar.copy(o_t[:, 512:1024], pr1[:])
                    nc.sync.dma_start(out_d[tb2 * 128:(tb2 + 1) * 128, :], o_t[:])

    nc.compile()
    return nc


def _host_prep(x, W_in, b_in, W_out):
    """Build per-core input maps."""
    import ml_dtypes

    bf = ml_dtypes.bfloat16
    perm = np.concatenate([np.arange(0, D, 2), np.arange(1, D, 2)])  # de-interleave
    inv_freq = 1.0 / (ROPE_BASE ** (np.arange(0, D, 2, dtype=np.float64) / D))
    tpos = np.arange(T, dtype=np.float64)
    freqs = np.outer(tpos, inv_freq)                   # [T, 32]
    cosw = np.cos(freqs).astype(np.float32)
    sinw = np.sin(freqs).astype(np.float32)
    cos2 = np.concatenate([cosw, cosw], axis=1)        # [T, 64]
    sin2 = np.concatenate([-sinw, sinw], axis=1)       # [T, 64]

    tri = (np.arange(128)[None, :] >= np.arange(128)[:, None]).astype(bf)
    ident = np.eye(128, dtype=bf)
    ones64 = np.ones((128, 64), dtype=bf)

    in_maps = []
    for c in range(N_CORES):
        b, half = c // 2, c % 2
        heads = np.arange(half * HPC, (half + 1) * HPC)

        xt = np.concatenate([np.ascontiguousarray(x[b].T),
                             np.ones((1, T), np.float32)], axis=0)

        qcols = np.concatenate([h * D + perm for h in heads])
        kcols = np.concatenate([C + h * D + perm for h in heads])
        vcols = np.concatenate([2 * C + h * D + np.arange(D) for h in heads])
        wqk = np.concatenate([W_in[:, np.concatenate([qcols, kcols])],
                              b_in[None, np.concatenate([qcols, kcols])]], axis=0)
        wv = np.concatenate([W_in[:, vcols], b_in[None, vcols]], axis=0)

        # flat row in ag_out[pair] [8, 130, 2048] viewed [8*130*2, 1024]:
        # (rank*130 + r)*2 + myhalf
        gidx = np.empty((128, 8), np.int32)
        gsidx = np.empty((128, 8), np.int32)
        p_arange = np.arange(128)
        for k in range(8):
            rank = 2 * b + k // 4
            gidx[:, k] = (rank * 130 + p_arange) * 2 + half
            gsidx[:, k] = (rank * 130 + 128 + p_arange // 64) * 2 + half

        in_maps.append({
            "xt": np.ascontiguousarray(xt),
            "wqk": np.ascontiguousarray(wqk.astype(np.float32)),
            "wv": np.ascontiguousarray(wv.astype(np.float32)),
            "wout": np.ascontiguousarray(W_out.astype(np.float32)),
            "cos2": cos2, "sin2": sin2,
            "trimask": tri, "ident": ident, "ones64": ones64,
            "gidx": gidx, "gsidx": gsidx,
        })
    return in_maps


LAST_RESULT = None


def kernel(x, W_in, b_in, W_out, b_out, _trace=False):
    global LAST_RESULT
    from concourse.bass_utils import run_bass_kernel_spmd

    x = np.asarray(x, dtype=np.float32)
    W_in = np.asarray(W_in, dtype=np.float32)
    b_in = np.asarray(b_in, dtype=np.float32)
    W_out = np.asarray(W_out, dtype=np.float32)
    b_out = np.asarray(b_out, dtype=np.float32)

    if "nc" not in _CACHE:
        _CACHE["nc"] = _build_program()
    nc = _CACHE["nc"]

    in_maps = _host_prep(x, W_in, b_in, W_out)
    res = run_bass_kernel_spmd(nc, in_maps, core_ids=list(range(N_CORES)),
                               trace=_trace)
    LAST_RESULT = res

    out = np.empty((B, T, C), np.float32)
    for c in range(N_CORES):
        b, half = c // 2, c % 2
        out[b, half * THALF:(half + 1) * THALF, :] = res.results[c]["out"]
    if np.any(b_out != 0):
        out = out + b_out[None, None, :]
    return out
